# revision 1
# baseline (speedup 1.0000x reference)
"""nn_HashMapper Trainium2 kernel (8 NeuronCores, Bass/Tile) — v4.

Contract: kernel(**inputs) takes the FULL unsharded inputs
(bits [32768,1024] i32, tables [3,1024,16384] f32, positions [3,14] i32)
and returns the FULL output [32768,1024] u8.

Sharding (hardcoded): neurons j (1024) split across 8 cores (128 each) so
tables are read exactly once system-wide; batch split across cores for
address computation; the tiny wrapped-address tensor is AllGather'd.

Engine layout (compute and DMA ride separate per-engine queues):
  - SP:   bits loads (even bt) | wrapped-addr writeback | tT0 writes |
          idx loads | tT2 tail writes | out writes
  - ACT:  bits loads (odd bt) | trb copies (odd bt) | tT1 writes |
          tT2 head writes
  - DVE:  bits casts + trb copies (even bt) + addr copies | all table
          stage copies | votes
  - PE:   bits transposes + addr matmuls | table transposes h0,h1,h2
  - Pool: tslice cast-loads | AllGather | gathers (SWDGE)
Key tricks:
  - tT rows bit-permuted (row m = g*2048 + a*16 + t for addr bits
    (g,t,a)) so staged tT writes are 4KB-contiguous runs (16x fewer
    descriptors); the permutation folds into the matmul weights for free.
  - addresses written in the gather's wrapped [16, b/16] layout before
    the AllGather, so idx loads are 512B-contiguous and the 128-partition
    replication is a stride-0 repeat dim on the DRAM source.
  - per-h tT tensors + per-h idx so gathers pipeline h0 -> h1 -> h2 with
    6/3/3 rotating slots; votes (2 adds + is_ge -> u8) and per-chunk out
    writes trail the gather stream.
"""

from contextlib import ExitStack

import numpy as np

import concourse.bass as bass
import concourse.bacc as bacc
import concourse.tile as tile
import concourse.mybir as mybir
from concourse.masks import make_identity
from concourse.bass_utils import run_bass_kernel_spmd

F32 = mybir.dt.float32
BF16 = mybir.dt.bfloat16
I32 = mybir.dt.int32
I16 = mybir.dt.int16
U8 = mybir.dt.uint8

N_BITS = 1024
NE = 16384
H = 3
JS = 128
B_TOTAL = 32768
N_CORES = 8

CHUNK = 2048  # gather chunk (batch rows per dma_gather)
CC = CHUNK // 128
NCK = B_TOTAL // CHUNK  # 16
S0 = 6  # h0 gather slots
S12 = 3  # h1/h2 gather slots
TGRP = 16  # table-transpose blocks per stage tile / tT write group


def _build(n_cores=N_CORES, nq=4):
    bsh = B_TOTAL // n_cores  # 4096 batch rows per core
    nbt = bsh // 128  # 32 b-tiles
    ncols = bsh // 16  # 256
    use_cc = n_cores > 1

    nc = bacc.Bacc(
        "TRN2", target_bir_lowering=False, num_devices=n_cores, num_swdge_queues=nq
    )
    bits = nc.dram_tensor("bits", [bsh, N_BITS], I32, kind="ExternalInput")
    tslice = nc.dram_tensor("tslice", [H, JS, NE], F32, kind="ExternalInput")
    w = nc.dram_tensor("w", [N_BITS, H], BF16, kind="ExternalInput")
    out = nc.dram_tensor("out", [B_TOTAL, JS], U8, kind="ExternalOutput")

    # wrapped addresses: [h, r=b%16, c=b//16] i16 (per-core c' = 0..255)
    addrw_loc = nc.dram_tensor("addrw_loc", [H, 16, ncols], I16)
    addrw_all = (
        nc.dram_tensor("addrw_all", [n_cores, H, 16, ncols], I16)
        if use_cc
        else addrw_loc
    )
    tT = [nc.dram_tensor(f"tT{h}", [NE, JS], BF16) for h in range(H)]

    with tile.TileContext(nc) as tc, ExitStack() as ctx:
        const = ctx.enter_context(tc.tile_pool(name="const", bufs=1))
        psT = ctx.enter_context(tc.tile_pool(name="psT", bufs=2, space="PSUM"))
        psB = ctx.enter_context(tc.tile_pool(name="psB", bufs=2, space="PSUM"))
        psA = ctx.enter_context(tc.tile_pool(name="psA", bufs=2, space="PSUM"))
        sbT = ctx.enter_context(tc.tile_pool(name="sbT", bufs=2))
        sbS = ctx.enter_context(tc.tile_pool(name="sbS", bufs=3))
        sbB = ctx.enter_context(tc.tile_pool(name="sbB", bufs=2))
        sbG = ctx.enter_context(tc.tile_pool(name="sbG", bufs=1))

        wsb = const.tile([128, 8, H], BF16)
        nc.sync.dma_start(wsb[:, :, :], w.rearrange("(kc p) h -> p kc h", p=128))
        ident = const.tile([128, 128], BF16)
        make_identity(nc, ident[:, :])
        addr16 = const.tile([128, nbt, H], I16)

        # ---- bits loads first (SP even bt, ACT odd bt, 8 rotating bufs) ----
        b32s = []
        for bt in range(nbt):
            t32 = sbB.tile([128, N_BITS], I32, tag="bits32", bufs=6)
            b32s.append(t32)
            eng = nc.sync if bt % 2 == 0 else nc.scalar
            eng.dma_start(t32[:], bits[bt * 128 : (bt + 1) * 128, :])

        # ---- tslice cast-loads (Pool) ----
        tsls = {}
        for h in range(H):
            for half in range(2):
                t = sbT.tile([128, NE // 2], BF16, tag=f"tsl{(h * 2 + half) % 3}",
                             bufs=1, name=f"tsl_{h}_{half}")
                nc.gpsimd.dma_start(
                    t[:], tslice[h, :, half * (NE // 2) : (half + 1) * (NE // 2)]
                )
                tsls[(h, half)] = t

        # ---- addr pipeline (casts on DVE; trb copies split DVE/ACT) ----
        for bt in range(nbt):
            tbf = sbB.tile([128, N_BITS], BF16, tag="bitsbf")
            nc.vector.tensor_copy(tbf[:], b32s[bt][:])
            pb = psB.tile([128, 8, 128], BF16, tag="trbits")
            for kc in range(8):
                nc.tensor.transpose(
                    pb[:, kc, :], tbf[:, kc * 128 : (kc + 1) * 128], ident[:, :]
                )
            trb = sbB.tile([128, 8, 128], BF16, tag="trb", bufs=3)
            if bt % 2 == 0:
                nc.vector.tensor_copy(trb[:, :, :], pb[:, :, :])
            else:
                nc.scalar.activation(
                    trb[:, :, :], pb[:, :, :], mybir.ActivationFunctionType.Copy
                )
            pa = psA.tile([128, H], F32, tag="addr")
            for kc in range(8):
                nc.tensor.matmul(
                    pa[:, :],
                    trb[:, kc, :],
                    wsb[:, kc, :],
                    start=(kc == 0),
                    stop=(kc == 7),
                )
            nc.vector.tensor_copy(addr16[:, bt, :], pa[:, :])

        # wrapped-layout writeback (per h): p=(g slow, r fast), bt ->
        # elem offset h*4096 + r*256 + bt*8 + g
        for h in range(H):
            nc.sync.dma_start(
                bass.AP(addrw_loc, h * 16 * ncols, [[1, 8], [ncols, 16], [8, nbt]]),
                addr16[:, :, h],
            )

        # ---- AllGather (gpsimd-only) ----
        if use_cc:
            nc.gpsimd.collective_compute(
                "AllGather",
                mybir.AluOpType.bypass,
                replica_groups=[list(range(n_cores))],
                ins=[addrw_loc.ap().opt()],
                outs=[addrw_all.ap().opt()],
            )

        # ---- table pipeline per h (PE transposes; copies/writes on
        #      h0: ACT+SP, h1: ACT+ACT, h2: DVE+DVE) ----
        h2_stages = []

        def table_h(h, write_eng):
            for g in range(NE // 128 // TGRP):  # 8 groups of 16 blocks
                pt = psT.tile([128, TGRP, 128], BF16, tag="trtab")
                for t in range(TGRP):
                    blk = g * TGRP + t
                    half, col = divmod(blk, (NE // 2) // 128)
                    nc.tensor.transpose(
                        pt[:, t, :],
                        tsls[(h, half)][:, col * 128 : (col + 1) * 128],
                        ident[:, :],
                    )
                stage = sbS.tile(
                    [128, TGRP, 128], BF16,
                    tag="stage" if write_eng is not None else f"stg2_{g}",
                    bufs=4 if write_eng is not None else 1,
                )
                nc.vector.tensor_copy(stage[:, :, :], pt[:, :, :])
                # permuted rows: memory row (within group g) = a*16 + t
                dst = tT[h][g * TGRP * 128 : (g + 1) * TGRP * 128, :].rearrange(
                    "(a t) j -> a t j", a=128
                )
                if write_eng is not None:
                    write_eng.dma_start(dst, stage[:, :, :])
                else:
                    h2_stages.append((dst, stage))

        # stage copies all on DVE; writes: h0 on SP, h1 on ACT,
        # h2 split ACT (first 4) + SP after the idx loads (last 4)
        table_h(0, nc.sync)
        table_h(1, nc.scalar)
        table_h(2, None)
        for dst, stage in h2_stages[:4]:
            nc.scalar.dma_start(dst, stage[:, :, :])

        # ---- idx loads: all 128 partitions directly via a stride-0
        #      repeat dim on the DRAM source (one DMA per (h, core)) ----
        idxs = []
        gcols = B_TOTAL // 16  # 2048
        for h in range(H):
            it = const.tile([128, gcols], I16, tag=f"idx{h}")
            for c in range(n_cores if use_cc else 1):
                base = addrw_all if use_cc else addrw_loc
                off = (c * H if use_cc else 0) * 16 * ncols + h * 16 * ncols
                nc.sync.dma_start(
                    it[:, c * ncols : (c + 1) * ncols],
                    bass.AP(base, off, [[0, 8], [ncols, 16], [1, ncols]]),
                )
            idxs.append(it)
        for dst, stage in h2_stages[4:]:
            nc.sync.dma_start(dst, stage[:, :, :])

        # ---- gather + votes + out (hand-synced critical section) ----
        gt0 = [sbG.tile([128, CC, JS], BF16, tag=f"g0s{s}", bufs=1, name=f"g0s{s}")
               for s in range(S0)]
        gt1 = [sbG.tile([128, CC, JS], BF16, tag=f"g1s{s}", bufs=1, name=f"g1s{s}")
               for s in range(S12)]
        gt2 = [sbG.tile([128, CC, JS], BF16, tag=f"g2s{s}", bufs=1, name=f"g2s{s}")
               for s in range(S12)]
        vtmp = [sbG.tile([128, CC, JS], BF16, tag=f"vt{s}", bufs=1, name=f"vt{s}")
                for s in range(2)]
        ots = [sbG.tile([128, CC, JS], U8, tag=f"os{s}", bufs=1, name=f"os{s}")
               for s in range(2)]
        gsem = [[nc.alloc_semaphore(f"gs{k}_{h}") for h in range(H)]
                for k in range(NCK)]
        vr01 = nc.alloc_semaphore("vr01")
        vr2 = nc.alloc_semaphore("vr2")
        vdone = nc.alloc_semaphore("vdone")
        osems = [nc.alloc_semaphore(f"osem{s}") for s in range(2)]
        outv = out.rearrange("(k cc p) j -> k p cc j", p=128, cc=CC)

        qn = [0]

        def gather(h, k, slot_tile):
            q = qn[0] % nq
            qn[0] += 1
            nc.gpsimd.dma_gather(
                slot_tile[:, :, :],
                tT[h][:, :],
                idxs[h][:, k * (CHUNK // 16) : (k + 1) * (CHUNK // 16)],
                num_idxs=CHUNK,
                num_idxs_reg=CHUNK,
                elem_size=JS,
                single_packet=False,
                queue_num=q,
            ).then_inc(gsem[k][h], 16)

        with tc.tile_critical():
            # ---- Pool: gathers (prefetch h0 x S0; then per chunk:
            #      h1, h2, and an h0 refill 2 chunks back of the horizon) ----
            for k in range(S0):
                gather(0, k, gt0[k % S0])
            for k in range(NCK):
                if k >= S12:
                    nc.gpsimd.wait_ge(vr01, k - S12 + 1)
                gather(1, k, gt1[k % S12])
                if k >= S12:
                    nc.gpsimd.wait_ge(vr2, k - S12 + 1)
                gather(2, k, gt2[k % S12])
                kk = k + S0 - 2
                if k >= 2 and kk < NCK:
                    nc.gpsimd.wait_ge(vr01, kk - S0 + 1)
                    gather(0, kk, gt0[kk % S0])
            # ---- DVE: votes ----
            for k in range(NCK):
                for h in range(H):
                    nc.vector.wait_ge(gsem[k][h], 16)
                if k >= 2:
                    nc.vector.wait_ge(vdone, k - 1)
                nc.vector.tensor_add(
                    vtmp[k % 2][:], gt0[k % S0][:], gt1[k % S12][:]
                ).then_inc(vr01, 1)
                nc.vector.wait_ge(vr01, k + 1)
                nc.vector.tensor_add(
                    vtmp[k % 2][:], vtmp[k % 2][:], gt2[k % S12][:]
                ).then_inc(vr2, 1)
                nc.vector.wait_ge(vr2, k + 1)
                if k >= 2:
                    nc.vector.wait_ge(osems[k % 2], 16 * ((k - 2) // 2 + 1))
                nc.vector.tensor_scalar(
                    ots[k % 2][:], vtmp[k % 2][:], 1.5, None,
                    op0=mybir.AluOpType.is_ge,
                ).then_inc(vdone, 1)
            # ---- SP: out writes ----
            for k in range(NCK):
                nc.sync.wait_ge(vdone, k + 1)
                nc.sync.dma_start(outv[k], ots[k % 2][:, :, :]).then_inc(
                    osems[k % 2], 16
                )
            for s in range(2):
                nc.sync.wait_ge(osems[s], 16 * (NCK // 2))

    nc.compile()
    return nc


def _make_w(positions):
    """Weights mapping bits -> bit-permuted tT row index.

    addr bit b contributes 2^b; the permuted row m = g*2048 + a*16 + t
    relabels addr bits (g=b13..11, t=b10..7, a=b6..0) to
    (g=m13..11, a=m10..4, t=m3..0).
    """
    import ml_dtypes

    def permbit(b):
        if b >= 11:
            return b
        if b >= 7:
            return b - 7
        return b + 4

    w = np.zeros((N_BITS, H), dtype=np.float32)
    for h in range(H):
        for kk in range(14):
            b = 13 - kk
            w[N_BITS - 1 - positions[h, kk], h] += 2.0 ** permbit(b)
    return w.astype(ml_dtypes.bfloat16)


_NC_CACHE = {}


def _get_nc():
    if "nc" not in _NC_CACHE:
        _NC_CACHE["nc"] = _build()
    return _NC_CACHE["nc"]


OUT_NAMES = ["out"]


def _make_in_maps(inputs):
    bits = np.ascontiguousarray(np.asarray(inputs["bits"], dtype=np.int32))
    tables = np.ascontiguousarray(np.asarray(inputs["tables"], dtype=np.float32))
    positions = np.asarray(inputs["positions"], dtype=np.int32)
    wnp = _make_w(positions)
    bsh = B_TOTAL // N_CORES
    return [
        {
            "bits": np.ascontiguousarray(bits[c * bsh : (c + 1) * bsh]),
            "tslice": np.ascontiguousarray(tables[:, c * JS : (c + 1) * JS, :]),
            "w": wnp,
        }
        for c in range(N_CORES)
    ]


def _assemble(outs, inputs):
    return np.concatenate([o["out"] for o in outs], axis=1)


def kernel(bits, tables, positions):
    nc = _get_nc()
    in_maps = _make_in_maps(
        {"bits": bits, "tables": tables, "positions": positions}
    )
    res = run_bass_kernel_spmd(nc, in_maps, core_ids=list(range(N_CORES)))
    return _assemble(res.results, None)



# revision 5
# speedup vs baseline: 2.1172x; 2.1172x over previous
"""nn_HashMapper Trainium2 kernel (8 NeuronCores, Bass/Tile) — v5.

Contract: kernel(**inputs) takes the FULL unsharded inputs
(bits [32768,1024] i32, tables [3,1024,16384] f32, positions [3,14] i32)
and returns the FULL output [32768,1024] u8.

Sharding (hardcoded): neurons j (1024) split across 8 cores (128 each) so
tables are read exactly once system-wide; batch split across cores for
address computation; the tiny wrapped-address tensor is AllGather'd.

v5 architecture — 2-bit packed tables:
  - Each core packs its 128 neurons' table values (0/1) into 2-bit fields
    of u16 words via PE matmuls (lhsT = tslice [j, addr-block], rhs = a
    128x16 weight of 4^(j%8)), 8 neurons/word, 16 words (32B) per address.
  - Packed rows are padded to 256B in DRAM (gather elem floor); gathers
    run with the table bitcast to int64 (elem 32 x i64) which quarters the
    per-index cost versus the bf16 layout.
  - Votes: gathered words of the 3 hash tables are summed as u16 (2-bit
    fields hold 0..3, no carries); the majority bit is bit1 of each field,
    extracted with 4 tensor_scalar (shift,and-0x0101) passes that emit two
    output bytes per u16 — out columns land in a (k,w,pair) order that the
    host inverse-permutes during unsharding.
  - Engine layout: SP: tslice h0 (f32) | PT writes h0,h2 | addrw/idx |
    half the out writes.  ACT: tslice h1 (f32) | PT write h1 | psum copies
    (odd) | other half of out writes.  Pool: tslice h2 (bf16 cast-load) |
    bitsel cast-load | AllGather | gathers.  DVE: psum copies (even) |
    addr conversion | votes + unpack.  PE: pack + address matmuls.
"""

from contextlib import ExitStack

import numpy as np

import concourse.bass as bass
import concourse.bacc as bacc
import concourse.tile as tile
import concourse.mybir as mybir
from concourse.bass_utils import run_bass_kernel_spmd

F32 = mybir.dt.float32
BF16 = mybir.dt.bfloat16
I32 = mybir.dt.int32
I16 = mybir.dt.int16
U16 = mybir.dt.uint16
I64 = mybir.dt.int64
U8 = mybir.dt.uint8
A = mybir.AluOpType

N_BITS = 1024
NE = 16384
H = 3
JS = 128
B_TOTAL = 32768
N_CORES = 8
NSEL = H * 14  # 42 selected bit columns

NW = 16         # u16 words per packed row (8 neurons each, 2-bit fields)
ROW_U16 = 128   # padded row size in u16 (256B, gather elem floor)
CHUNK = 2048    # gather chunk (batch rows per dma_gather)
CC = CHUNK // 128
NCK = B_TOTAL // CHUNK  # 16
S = 3           # gather slots per h


def _build(n_cores=N_CORES, nq=4):
    bsh = B_TOTAL // n_cores  # 4096 batch rows per core
    nbt = bsh // 128  # 32 b-tiles
    ncols = bsh // 16  # 256
    use_cc = n_cores > 1

    nc = bacc.Bacc(
        "TRN2", target_bir_lowering=False, num_devices=n_cores, num_swdge_queues=nq
    )
    tslice = nc.dram_tensor("tslice", [H, JS, NE], F32, kind="ExternalInput")
    bitsel = nc.dram_tensor("bitsel", [NSEL, bsh], I32, kind="ExternalInput")
    waddr = nc.dram_tensor("waddr", [NSEL, H], BF16, kind="ExternalInput")
    wpackb = nc.dram_tensor("wpackb", [JS, NW], BF16, kind="ExternalInput")
    wpackf = nc.dram_tensor("wpackf", [JS, NW], F32, kind="ExternalInput")
    out = nc.dram_tensor("out", [B_TOTAL, JS], U8, kind="ExternalOutput")

    # wrapped addresses: [h, r=b%16, c=b//16] i16 (per-core c' = 0..255)
    addrw_loc = nc.dram_tensor("addrw_loc", [H, 16, ncols], I16)
    addrw_all = (
        nc.dram_tensor("addrw_all", [n_cores, H, 16, ncols], I16)
        if use_cc
        else addrw_loc
    )
    # packed table, padded rows: row (h, a) = PT[h, a, 0:NW] words + pad
    PT = nc.dram_tensor("PT", [H, NE, ROW_U16], U16)

    with tile.TileContext(nc) as tc, ExitStack() as ctx:
        const = ctx.enter_context(tc.tile_pool(name="const", bufs=1))
        psT = ctx.enter_context(tc.tile_pool(name="psT", bufs=4, space="PSUM"))
        psA = ctx.enter_context(tc.tile_pool(name="psA", bufs=1, space="PSUM"))
        sbT = ctx.enter_context(tc.tile_pool(name="sbT", bufs=2))
        sbP = ctx.enter_context(tc.tile_pool(name="sbP", bufs=1))
        sbG = ctx.enter_context(tc.tile_pool(name="sbG", bufs=1))

        wpb = const.tile([JS, NW], BF16)
        nc.sync.dma_start(wpb[:, :], wpackb[:, :])
        wpf = const.tile([JS, NW], F32)
        nc.sync.dma_start(wpf[:, :], wpackf[:, :])
        wad = const.tile([NSEL, H], BF16)
        nc.sync.dma_start(wad[:, :], waddr[:, :])

        # ---- bitsel cast-load (Pool, i32 -> bf16) ----
        bsl = const.tile([NSEL, bsh], BF16)
        nc.gpsimd.dma_start(bsl[:, :], bitsel[:, :])

        # ---- address matmuls (PE) -> psA -> addr16 (DVE) ----
        pa = psA.tile([128, nbt, H], F32, tag="addr")
        for bt in range(nbt):
            nc.tensor.matmul(
                pa[:, bt, :],
                bsl[:, bt * 128 : (bt + 1) * 128],
                wad[:, :],
                start=True,
                stop=True,
            )
        addr16 = const.tile([128, nbt, H], I16)
        nc.vector.tensor_copy(addr16[:, :, :], pa[:, :, :])

        # wrapped-layout writeback (per h): p=(g slow, r fast), bt ->
        # elem offset h*4096 + r*256 + bt*8 + g
        for h in range(H):
            nc.sync.dma_start(
                bass.AP(addrw_loc, h * 16 * ncols, [[1, 8], [ncols, 16], [8, nbt]]),
                addr16[:, :, h],
            )

        # ---- AllGather (gpsimd-only) ----
        if use_cc:
            nc.gpsimd.collective_compute(
                "AllGather",
                mybir.AluOpType.bypass,
                replica_groups=[list(range(n_cores))],
                ins=[addrw_loc.ap().opt()],
                outs=[addrw_all.ap().opt()],
            )

        # ---- idx loads: per (h, core) replicated reads (stride-0 repeat) ----
        gcols = B_TOTAL // 16  # 2048
        idxs = []
        for h in range(H):
            it = const.tile([128, gcols], I16, tag=f"idx{h}")
            for c in range(n_cores if use_cc else 1):
                base = addrw_all if use_cc else addrw_loc
                off = (c * H if use_cc else 0) * 16 * ncols + h * 16 * ncols
                eng = nc.sync if (h * n_cores + c) % 2 == 0 else nc.scalar
                eng.dma_start(
                    it[:, c * ncols : (c + 1) * ncols],
                    bass.AP(base, off, [[0, 8], [ncols, 16], [1, ncols]]),
                )
            idxs.append(it)

        # ---- tslice load + pack (PE matmuls) + PT writes ----
        # h0 -> SP (f32), h1 -> ACT (f32), h2 -> Pool (bf16 cast-load)
        TL = 4096  # tslice tile columns (addr)
        pts = [const.tile([128, NE // 128, NW], U16, name=f"pts{h}") for h in range(H)]
        cp_i = [0]
        for h, (eng, dt_, wt) in enumerate(
            [(nc.sync, F32, None), (nc.scalar, F32, None), (nc.gpsimd, BF16, None)]
        ):
            wt = wpf if dt_ == F32 else wpb
            for t in range(NE // TL):
                tsl = sbT.tile([128, TL], dt_, tag=f"tsl{h % 2}", bufs=3,
                               name=f"tsl_{h}_{t}")
                eng.dma_start(tsl[:, :], tslice[h, :, t * TL : (t + 1) * TL])
                for bg in range(TL // 128 // 16):  # 2 groups of 16 blocks
                    ps = psT.tile([128, 16, NW], F32, tag="pack")
                    for b16 in range(16):
                        blk = bg * 16 + b16
                        nc.tensor.matmul(
                            ps[:, b16, :],
                            tsl[:, blk * 128 : (blk + 1) * 128],
                            wt[:, :],
                            start=True,
                            stop=True,
                        )
                    r0 = t * (TL // 128) + bg * 16
                    ceng = nc.vector if cp_i[0] % 2 == 0 else nc.scalar
                    cp_i[0] += 1
                    if ceng is nc.vector:
                        ceng.tensor_copy(pts[h][:, r0 : r0 + 16, :], ps[:, :, :])
                    else:
                        ceng.activation(
                            pts[h][:, r0 : r0 + 16, :], ps[:, :, :],
                            mybir.ActivationFunctionType.Copy,
                        )
            # PT write for this h: rows a = rank*128 + p, words 0:NW
            weng = [nc.sync, nc.scalar, nc.sync][h]
            weng.dma_start(
                bass.AP(
                    PT,
                    h * NE * ROW_U16,
                    [[ROW_U16, 128], [128 * ROW_U16, NE // 128], [1, NW]],
                ),
                pts[h][:, :, :],
            )

        # ---- gather + votes + out (hand-synced critical section) ----
        gts = [
            [sbG.tile([128, CC, ROW_U16 // 4], I64, tag=f"g{h}s{s}", bufs=1,
                      name=f"g{h}s{s}") for s in range(S)]
            for h in range(H)
        ]
        vts = [sbG.tile([128, CC, NW], U16, tag=f"vt{s}", bufs=1, name=f"vt{s}")
               for s in range(2)]
        ots = [sbG.tile([128, CC, 4, NW], U16, tag=f"os{s}", bufs=1, name=f"os{s}")
               for s in range(2)]
        gsem = [[nc.alloc_semaphore(f"gs{k}_{h}") for h in range(H)]
                for k in range(NCK)]
        vr01 = nc.alloc_semaphore("vr01")
        vr2 = nc.alloc_semaphore("vr2")
        vdone = nc.alloc_semaphore("vdone")
        osems = [nc.alloc_semaphore(f"osem{s}") for s in range(2)]

        qn = [0]

        def gather(h, k):
            q = qn[0] % nq
            qn[0] += 1
            nc.gpsimd.dma_gather(
                gts[h][k % S][:, :, :],
                bass.AP(PT, h * NE * ROW_U16, [[ROW_U16, NE], [1, ROW_U16]]).bitcast(
                    I64
                ),
                idxs[h][:, k * (CHUNK // 16) : (k + 1) * (CHUNK // 16)],
                num_idxs=CHUNK,
                num_idxs_reg=CHUNK,
                elem_size=ROW_U16 // 4,
                single_packet=False,
                queue_num=q,
            ).then_inc(gsem[k][h], 16)

        with tc.tile_critical():
            # ---- Pool: gathers ----
            for k in range(NCK):
                if k >= S:
                    nc.gpsimd.wait_ge(vr01, k - S + 1)
                gather(0, k)
                gather(1, k)
                if k >= S:
                    nc.gpsimd.wait_ge(vr2, k - S + 1)
                gather(2, k)
            # ---- DVE: votes + unpack (4 passes inc vdone each) ----
            for k in range(NCK):
                for h in range(H):
                    nc.vector.wait_ge(gsem[k][h], 16)
                if k >= 2:
                    nc.vector.wait_ge(vdone, 4 * (k - 1))
                g0 = gts[0][k % S][:, :, :].bitcast(U16)[:, :, 0:NW]
                g1 = gts[1][k % S][:, :, :].bitcast(U16)[:, :, 0:NW]
                g2 = gts[2][k % S][:, :, :].bitcast(U16)[:, :, 0:NW]
                vt = vts[k % 2]
                nc.vector.tensor_tensor(vt[:, :, :], g0, g1, op=A.add).then_inc(
                    vr01, 1
                )
                nc.vector.wait_ge(vr01, k + 1)
                nc.vector.tensor_tensor(
                    vt[:, :, :], vt[:, :, :], g2, op=A.add
                ).then_inc(vr2, 1)
                nc.vector.wait_ge(vr2, k + 1)
                if k >= 2:
                    nc.vector.wait_ge(osems[k % 2], 16 * ((k - 2) // 2 + 1))
                ot = ots[k % 2]
                for kk in range(4):
                    nc.vector.tensor_scalar(
                        ot[:, :, kk, :], vt[:, :, :], 2 * kk + 1, 0x0101,
                        op0=A.logical_shift_right, op1=A.bitwise_and,
                    ).then_inc(vdone, 1)
            # ---- SP/ACT: out writes ----
            for k in range(NCK):
                eng = nc.sync if k % 2 == 0 else nc.scalar
                eng.wait_ge(vdone, 4 * (k + 1))
                eng.dma_start(
                    bass.AP(out, k * CHUNK * JS, [[JS, 128], [128 * JS, CC], [1, JS]]),
                    ots[k % 2][:, :, :, :].bitcast(U8),
                ).then_inc(osems[k % 2], 16)
            for s in range(2):
                nc.sync.wait_ge(osems[s], 16 * (NCK // 2))
                nc.scalar.wait_ge(osems[s], 16 * (NCK // 2))

    nc.compile()
    return nc


def _make_weights(positions):
    """Host-side tiny weight tensors derived from positions."""
    import ml_dtypes

    waddr = np.zeros((NSEL, H), dtype=np.float32)
    for h in range(H):
        for k in range(14):
            waddr[h * 14 + k, h] = float(1 << (13 - k))
    wpack = np.zeros((JS, NW), dtype=np.float32)
    for jl in range(JS):
        wpack[jl, jl // 8] = float(4 ** (jl % 8))
    return (
        waddr.astype(ml_dtypes.bfloat16),
        wpack.astype(ml_dtypes.bfloat16),
        wpack,
    )


_NC_CACHE = {}


def _get_nc():
    if "nc" not in _NC_CACHE:
        _NC_CACHE["nc"] = _build()
    return _NC_CACHE["nc"]


OUT_NAMES = ["out"]


def _col_unperm():
    """Map output column j_local -> device column (k*32 + w*2 + byte)."""
    dmap = np.empty(JS, dtype=np.int64)
    for jl in range(JS):
        w, r = jl // 8, jl % 8
        k, b = r % 4, r // 4
        dmap[jl] = k * 32 + w * 2 + b
    return dmap


def _make_in_maps(inputs):
    bits = np.asarray(inputs["bits"], dtype=np.int32)
    tables = np.ascontiguousarray(np.asarray(inputs["tables"], dtype=np.float32))
    positions = np.asarray(inputs["positions"], dtype=np.int32)
    wa, wpb, wpf = _make_weights(positions)
    cols = np.array(
        [N_BITS - 1 - positions[h, k] for h in range(H) for k in range(14)],
        dtype=np.int64,
    )
    bsh = B_TOTAL // N_CORES
    return [
        {
            "tslice": np.ascontiguousarray(tables[:, c * JS : (c + 1) * JS, :]),
            "bitsel": np.ascontiguousarray(
                bits[c * bsh : (c + 1) * bsh, cols].T
            ),
            "waddr": wa,
            "wpackb": wpb,
            "wpackf": wpf,
        }
        for c in range(N_CORES)
    ]


def _assemble(outs, inputs):
    dmap = _col_unperm()
    return np.concatenate([o["out"][:, dmap] for o in outs], axis=1)


def kernel(bits, tables, positions):
    nc = _get_nc()
    in_maps = _make_in_maps(
        {"bits": bits, "tables": tables, "positions": positions}
    )
    res = run_bass_kernel_spmd(nc, in_maps, core_ids=list(range(N_CORES)))
    return _assemble(res.results, None)


# revision 6
# speedup vs baseline: 2.6447x; 1.2491x over previous
"""nn_HashMapper Trainium2 kernel (8 NeuronCores, Bass/Tile) — v6.

Contract: kernel(**inputs) takes the FULL unsharded inputs
(bits [32768,1024] i32, tables [3,1024,16384] f32, positions [3,14] i32)
and returns the FULL output [32768,1024] u8.

Sharding (hardcoded): neurons j (1024) split across 8 cores (128 each) so
tables are read exactly once system-wide; every core computes the full
batch's hash addresses locally from a replicated 42-column slice of bits
(no cross-core communication at all).

v6 architecture — 2-bit packed tables, bf16 byte-view inputs:
  - tslice ships as the high 2 bytes of each f32 (exact for 0/1 values) so
    table loads cost half.
  - Each core packs its 128 neurons' table values into 2-bit fields of u16
    words via PE matmuls (lhsT = tslice [j, addr-block], rhs = 128x16
    weights of 4^(j%8)), 8 neurons/word, 16 words (32B) per address.
  - Packed rows are padded to 256B in DRAM; gathers run with the table
    bitcast to int64 (elem 32 x i64).
  - Addresses: bitsel64 packs the 42 selected bit-columns for each half of
    the batch into partitions [0:42] / [64:106]; 256 PE matmuls produce all
    32768 addresses; a wrapped DRAM round-trip builds the replicated
    16-partition index tiles.
  - Votes: gathered words of the 3 hash tables are summed as u16 (2-bit
    fields hold 0..3, no carries); majority = bit1 of each field, extracted
    with 4 tensor_scalar (shift, and-0x0101) passes emitting two output
    bytes per u16; the host inverse-permutes output columns.
"""

from contextlib import ExitStack

import numpy as np

import concourse.bass as bass
import concourse.bacc as bacc
import concourse.tile as tile
import concourse.mybir as mybir
from concourse.bass_utils import run_bass_kernel_spmd

F32 = mybir.dt.float32
BF16 = mybir.dt.bfloat16
I32 = mybir.dt.int32
I16 = mybir.dt.int16
U16 = mybir.dt.uint16
I64 = mybir.dt.int64
U8 = mybir.dt.uint8
A = mybir.AluOpType

N_BITS = 1024
NE = 16384
H = 3
JS = 128
B_TOTAL = 32768
N_CORES = 8
NSEL = H * 14  # 42 selected bit columns

NW = 16         # u16 words per packed row (8 neurons each, 2-bit fields)
ROW_U16 = 128   # padded row size in u16 (256B, gather elem floor)
CHUNK = 2048    # gather chunk (batch rows per dma_gather)
CC = CHUNK // 128
NCK = B_TOTAL // CHUNK  # 16
S = 3           # gather slots per h
NBT = B_TOTAL // 128  # 256 address blocks
GC = B_TOTAL // 16    # 2048 wrapped idx columns


def _build(n_cores=N_CORES, nq=4):
    nc = bacc.Bacc(
        "TRN2", target_bir_lowering=False, num_devices=n_cores, num_swdge_queues=nq
    )
    tslice = nc.dram_tensor("tslice", [H, JS, NE], BF16, kind="ExternalInput")
    bitsel = nc.dram_tensor("bitsel", [128, B_TOTAL // 2], I32, kind="ExternalInput")
    waddr = nc.dram_tensor("waddr", [128, H], BF16, kind="ExternalInput")
    wpack = nc.dram_tensor("wpack", [JS, NW], BF16, kind="ExternalInput")
    out = nc.dram_tensor("out", [B_TOTAL, JS], U8, kind="ExternalOutput")

    # wrapped addresses for the full batch: [h, r=b%16, c=b//16] i16
    addrw = nc.dram_tensor("addrw", [H, 16, GC], I16)
    # packed table, padded rows: row (h, a) = PT[h, a, 0:NW] words + pad
    PT = nc.dram_tensor("PT", [H, NE, ROW_U16], U16)

    with tile.TileContext(nc) as tc, ExitStack() as ctx:
        const = ctx.enter_context(tc.tile_pool(name="const", bufs=1))
        psT = ctx.enter_context(tc.tile_pool(name="psT", bufs=4, space="PSUM"))
        psA = ctx.enter_context(tc.tile_pool(name="psA", bufs=2, space="PSUM"))
        sbT = ctx.enter_context(tc.tile_pool(name="sbT", bufs=2))
        sbG = ctx.enter_context(tc.tile_pool(name="sbG", bufs=1))

        wpk = const.tile([JS, NW], BF16)
        nc.sync.dma_start(wpk[:, :], wpack[:, :])
        wad = const.tile([128, H], BF16)
        nc.sync.dma_start(wad[:, :], waddr[:, :])

        # ---- bitsel cast-load (Pool, i32 -> bf16), halves for pipelining ----
        bsl = const.tile([128, B_TOTAL // 2], BF16)
        nc.gpsimd.dma_start(bsl[:, 0 : B_TOTAL // 4], bitsel[:, 0 : B_TOTAL // 4])
        nc.gpsimd.dma_start(bsl[:, B_TOTAL // 4 :], bitsel[:, B_TOTAL // 4 :])

        # ---- address matmuls (PE): 256 blocks, two partition groups ----
        addr16 = const.tile([128, NBT, H], I16)
        for half in range(2):
            pa = psA.tile([128, NBT // 2, H], F32, tag="addr")
            for m in range(NBT // 2):
                g, blk = divmod(half * (NBT // 2) + m, NBT // 2)
                nc.tensor.matmul(
                    pa[:, m, :],
                    bsl[64 * g : 64 * g + NSEL, blk * 128 : (blk + 1) * 128],
                    wad[64 * g : 64 * g + NSEL, :],
                    start=True,
                    stop=True,
                )
            nc.vector.tensor_copy(
                addr16[:, half * (NBT // 2) : (half + 1) * (NBT // 2), :],
                pa[:, :, :],
            )

        # wrapped-layout writeback (per h): p=(g slow, r fast), bt ->
        # elem offset h*16*GC + r*GC + bt*8 + g
        for h in range(H):
            nc.sync.dma_start(
                bass.AP(addrw, h * 16 * GC, [[1, 8], [GC, 16], [8, NBT]]),
                addr16[:, :, h],
            )

        # ---- idx loads: per h, one replicated read (stride-0 repeat) ----
        idxs = []
        for h in range(H):
            it = const.tile([128, GC], I16, tag=f"idx{h}")
            nc.scalar.dma_start(
                it[:, :],
                bass.AP(addrw, h * 16 * GC, [[0, 8], [GC, 16], [1, GC]]),
            )
            idxs.append(it)

        # ---- tslice load + pack (PE matmuls) + PT writes ----
        # loads: h0 -> SP, h1 -> ACT, h2 -> Pool halves
        TL = 4096  # tslice tile columns (addr)
        pts = [const.tile([128, NE // 128, NW], U16, name=f"pts{h}") for h in range(H)]
        load_eng = {0: [nc.sync] * 4, 1: [nc.scalar] * 4,
                    2: [nc.gpsimd, nc.gpsimd, nc.gpsimd, nc.gpsimd]}
        for h in range(H):
            for t in range(NE // TL):
                tsl = sbT.tile([128, TL], BF16, tag=f"tsl{h % 2}", bufs=3,
                               name=f"tsl_{h}_{t}")
                load_eng[h][t].dma_start(
                    tsl[:, :], tslice[h, :, t * TL : (t + 1) * TL]
                )
                for bg in range(TL // 128 // 16):  # 2 groups of 16 blocks
                    ps = psT.tile([128, 16, NW], F32, tag="pack")
                    for b16 in range(16):
                        blk = bg * 16 + b16
                        nc.tensor.matmul(
                            ps[:, b16, :],
                            tsl[:, blk * 128 : (blk + 1) * 128],
                            wpk[:, :],
                            start=True,
                            stop=True,
                        )
                    r0 = t * (TL // 128) + bg * 16
                    nc.vector.tensor_copy(pts[h][:, r0 : r0 + 16, :], ps[:, :, :])
            # PT write for this h: rows a = rank*128 + p, words 0:NW
            weng = [nc.sync, nc.scalar, nc.scalar][h]
            weng.dma_start(
                bass.AP(
                    PT,
                    h * NE * ROW_U16,
                    [[ROW_U16, 128], [128 * ROW_U16, NE // 128], [1, NW]],
                ),
                pts[h][:, :, :],
            )

        # ---- gather + votes + out (hand-synced critical section) ----
        gts = [
            [sbG.tile([128, CC, ROW_U16 // 4], I64, tag=f"g{h}s{s}", bufs=1,
                      name=f"g{h}s{s}") for s in range(S)]
            for h in range(H)
        ]
        vts = [sbG.tile([128, CC, NW], U16, tag=f"vt{s}", bufs=1, name=f"vt{s}")
               for s in range(2)]
        ots = [sbG.tile([128, CC, 4, NW], U16, tag=f"os{s}", bufs=1, name=f"os{s}")
               for s in range(2)]
        gsem = [[nc.alloc_semaphore(f"gs{k}_{h}") for h in range(H)]
                for k in range(NCK)]
        vr01 = nc.alloc_semaphore("vr01")
        vr2 = nc.alloc_semaphore("vr2")
        vdone = nc.alloc_semaphore("vdone")
        osems = [nc.alloc_semaphore(f"osem{s}") for s in range(2)]

        qn = [0]

        def gather(h, k):
            q = qn[0] % nq
            qn[0] += 1
            nc.gpsimd.dma_gather(
                gts[h][k % S][:, :, :],
                bass.AP(PT, h * NE * ROW_U16, [[ROW_U16, NE], [1, ROW_U16]]).bitcast(
                    I64
                ),
                idxs[h][:, k * (CHUNK // 16) : (k + 1) * (CHUNK // 16)],
                num_idxs=CHUNK,
                num_idxs_reg=CHUNK,
                elem_size=ROW_U16 // 4,
                single_packet=False,
                queue_num=q,
            ).then_inc(gsem[k][h], 16)

        with tc.tile_critical():
            # ---- Pool: gathers ----
            for k in range(NCK):
                if k >= S:
                    nc.gpsimd.wait_ge(vr01, k - S + 1)
                gather(0, k)
                gather(1, k)
                if k >= S:
                    nc.gpsimd.wait_ge(vr2, k - S + 1)
                gather(2, k)
            # ---- DVE: votes + unpack (4 passes inc vdone each) ----
            for k in range(NCK):
                for h in range(H):
                    nc.vector.wait_ge(gsem[k][h], 16)
                if k >= 2:
                    nc.vector.wait_ge(vdone, 4 * (k - 1))
                g0 = gts[0][k % S][:, :, :].bitcast(U16)[:, :, 0:NW]
                g1 = gts[1][k % S][:, :, :].bitcast(U16)[:, :, 0:NW]
                g2 = gts[2][k % S][:, :, :].bitcast(U16)[:, :, 0:NW]
                vt = vts[k % 2]
                nc.vector.tensor_tensor(vt[:, :, :], g0, g1, op=A.add).then_inc(
                    vr01, 1
                )
                nc.vector.wait_ge(vr01, k + 1)
                nc.vector.tensor_tensor(
                    vt[:, :, :], vt[:, :, :], g2, op=A.add
                ).then_inc(vr2, 1)
                nc.vector.wait_ge(vr2, k + 1)
                if k >= 2:
                    nc.vector.wait_ge(osems[k % 2], 16 * ((k - 2) // 2 + 1))
                ot = ots[k % 2]
                for kk in range(4):
                    nc.vector.tensor_scalar(
                        ot[:, :, kk, :], vt[:, :, :], 2 * kk + 1, 0x0101,
                        op0=A.logical_shift_right, op1=A.bitwise_and,
                    ).then_inc(vdone, 1)
            # ---- SP/ACT: out writes ----
            for k in range(NCK):
                eng = nc.sync if k % 2 == 0 else nc.scalar
                eng.wait_ge(vdone, 4 * (k + 1))
                eng.dma_start(
                    bass.AP(out, k * CHUNK * JS, [[JS, 128], [128 * JS, CC], [1, JS]]),
                    ots[k % 2][:, :, :, :].bitcast(U8),
                ).then_inc(osems[k % 2], 16)
            for s in range(2):
                nc.sync.wait_ge(osems[s], 16 * (NCK // 2))
                nc.scalar.wait_ge(osems[s], 16 * (NCK // 2))

    nc.compile()
    return nc


def _make_weights(positions):
    """Host-side tiny weight tensors derived from positions."""
    import ml_dtypes

    waddr = np.zeros((128, H), dtype=np.float32)
    for h in range(H):
        for k in range(14):
            waddr[h * 14 + k, h] = float(1 << (13 - k))
            waddr[64 + h * 14 + k, h] = float(1 << (13 - k))
    wpack = np.zeros((JS, NW), dtype=np.float32)
    for jl in range(JS):
        wpack[jl, jl // 8] = float(4 ** (jl % 8))
    return (
        waddr.astype(ml_dtypes.bfloat16),
        wpack.astype(ml_dtypes.bfloat16),
    )


_NC_CACHE = {}


def _get_nc():
    if "nc" not in _NC_CACHE:
        _NC_CACHE["nc"] = _build()
    return _NC_CACHE["nc"]


OUT_NAMES = ["out"]


def _col_unperm():
    """Map output column j_local -> device column (k*32 + w*2 + byte)."""
    dmap = np.empty(JS, dtype=np.int64)
    for jl in range(JS):
        w, r = jl // 8, jl % 8
        k, b = r % 4, r // 4
        dmap[jl] = k * 32 + w * 2 + b
    return dmap


def _make_in_maps(inputs):
    import ml_dtypes

    bits = np.asarray(inputs["bits"], dtype=np.int32)
    tables = np.ascontiguousarray(np.asarray(inputs["tables"], dtype=np.float32))
    positions = np.asarray(inputs["positions"], dtype=np.int32)
    wa, wp = _make_weights(positions)
    cols = np.array(
        [N_BITS - 1 - positions[h, k] for h in range(H) for k in range(14)],
        dtype=np.int64,
    )
    # bf16 byte-view of the f32 tables (exact: values are 0.0/1.0)
    tb16 = tables.view(np.uint16)[:, :, 1::2]
    # bitsel64: 42 selected columns for each half of the batch, packed into
    # partitions [0:42] and [64:106]
    sel = bits[:, cols].T  # [42, B_TOTAL]
    b64 = np.zeros((128, B_TOTAL // 2), dtype=np.int32)
    b64[0:NSEL] = sel[:, : B_TOTAL // 2]
    b64[64 : 64 + NSEL] = sel[:, B_TOTAL // 2 :]
    return [
        {
            "tslice": np.ascontiguousarray(tb16[:, c * JS : (c + 1) * JS, :]).view(
                ml_dtypes.bfloat16
            ),
            "bitsel": b64,
            "waddr": wa,
            "wpack": wp,
        }
        for c in range(N_CORES)
    ]


def _assemble(outs, inputs):
    dmap = _col_unperm()
    return np.concatenate([o["out"][:, dmap] for o in outs], axis=1)


def kernel(bits, tables, positions):
    nc = _get_nc()
    in_maps = _make_in_maps(
        {"bits": bits, "tables": tables, "positions": positions}
    )
    res = run_bass_kernel_spmd(nc, in_maps, core_ids=list(range(N_CORES)))
    return _assemble(res.results, None)
